# revision 35
# baseline (speedup 1.0000x reference)
"""GCN layer (dense projection + sparse neighbor aggregation) on 8 Trainium2
NeuronCores via Bass/Tile.

Strategy: shard nodes (and their incident edges, grouped by destination row)
across the 8 cores; replicate W/b; AllGather the projected node features in
f32 (4 bucket-aligned sub-collectives pipelined with a bf16 projection); per
128-row output block, bulk-gather source rows with DMAGatherAnt (int16
indices into 4 sub-tables of <=32k rows), scale by edge_val on DVE, fold
J same-destination slots per lane with tensor_reduce (bucket-pure tiers:
J=5 x1 chunk for bucket 0, J=3 x2 chunks for buckets 1-3 => 7 one-hot
matmuls per block instead of 18+), and segment-sum via transposed
assignment-matrix matmuls accumulated in PSUM [64 feats x 128 dests];
bias+ReLU fused in one scalar activation per block.
"""

import sys

if "/opt/trn_rl_repo" not in sys.path:
    sys.path.insert(0, "/opt/trn_rl_repo")

import numpy as np

import concourse.bass as bass
import concourse.mybir as mybir
import concourse.tile as tile
from concourse import bacc
from concourse.bass_utils import run_bass_kernel_spmd

N_NODES = 100000
N_EDGES = 1600000
IN_FT = 256
OUT_FT = 64
NCORES = 8
NS = N_NODES // NCORES          # 12500 nodes per core
NB = (NS + 127) // 128          # 98 row blocks per core
NSP = NB * 128                  # 12544 padded nodes per core
GB = 7                          # row blocks per pipeline group (98 = 14 * 7)
NGROUPS = NB // GB              # 14
NSUB = 4                        # source-block buckets (int16 idx: <=32k rows)
QB = [16, 28, 27, 27]           # source blocks per bucket (sums to NB);
                                # small first bucket -> AllGather 0 fires early
QBS = np.concatenate([[0], np.cumsum(QB)]).astype(np.int64)
SUBROWS = [NCORES * 128 * q for q in QB]

F32 = mybir.dt.float32
F16 = mybir.dt.float16
BF16 = mybir.dt.bfloat16
I16 = mybir.dt.int16

MAXCH = 8                       # 1024 indices = HW cap per dma_gather
NQ = 4                          # SWDGE queues (ucode max)

# ---- bucket-pure fold tiers -------------------------------------------------
# cfg = (J per bucket, chunks per bucket). caps[b] = 128 * nchb[b] lanes.
CFG_LADDER = [
    ((5, 3, 3, 3), (1, 2, 2, 2)),
    ((6, 4, 4, 4), (1, 2, 2, 2)),
]


def cfg_geom(cfg):
    JB, NCHB = cfg
    colsb = [J * n for J, n in zip(JB, NCHB)]       # cols per bucket
    chb = np.concatenate([[0], np.cumsum(NCHB)])    # chunk offset per bucket
    gb_cols = [GB * cb for cb in colsb]             # group cols per bucket
    gbase = np.concatenate([[0], np.cumsum(gb_cols)])  # group col base
    nch = int(chb[-1])                              # chunks per block (7)
    sgt = int(gbase[-1])                            # group cols (161)
    return JB, NCHB, colsb, chb, gbase, nch, sgt


def build_program(cfg):
    """One SPMD Bass program; all 8 cores run it on their own shards."""
    JB, NCHB, colsb, chb, gbase, nch, sgt = cfg_geom(cfg)

    nc = bacc.Bacc("TRN2", target_bir_lowering=False, debug=False,
                   num_devices=NCORES, num_swdge_queues=NQ)

    seqT = nc.dram_tensor("seqT", [2, 128, NSP], BF16, kind="ExternalInput")
    gidx = nc.dram_tensor("gidx", [128, NGROUPS, sgt * 8], I16,
                          kind="ExternalInput")
    val = nc.dram_tensor("val", [128, NGROUPS, sgt], F16,
                         kind="ExternalInput")
    rl = nc.dram_tensor("rl", [128, NB, nch], F16, kind="ExternalInput")
    w_in = nc.dram_tensor("w", [128, 2, OUT_FT], BF16, kind="ExternalInput")
    bias_in = nc.dram_tensor("biasT", [OUT_FT, 1], F32, kind="ExternalInput")
    iota_in = nc.dram_tensor("iotat", [128, nch * 128], F16,
                             kind="ExternalInput")
    # partition-major layouts; host un-permutes
    sf_out = nc.dram_tensor("sf", [128, NB, OUT_FT], F16,
                            kind="ExternalOutput")
    ccin = [nc.dram_tensor(f"ccin{b}", [128, QB[b], OUT_FT], F16)
            for b in range(NSUB)]
    agg_out = nc.dram_tensor("aggT", [OUT_FT, NB * 128], F16,
                             kind="ExternalOutput")
    # fp16 AllGather output; upcast on-device into the f32 gather table
    # (dma_gather elements must be a multiple of 256B = 64 x f32)
    xfull16 = [nc.dram_tensor(f"xfull16_{b}", [SUBROWS[b], OUT_FT], F16,
                              addr_space="Shared") for b in range(NSUB)]
    xfull = [nc.dram_tensor(f"xfull{b}", [SUBROWS[b], OUT_FT], F32)
             for b in range(NSUB)]

    groups = [list(range(NCORES))]

    with tile.TileContext(nc) as tc:
        with (
            tc.tile_pool(name="const", bufs=1) as cpool,
            tc.tile_pool(name="psum", bufs=4, space="PSUM") as psum_pool,
            tc.tile_pool(name="meta", bufs=4) as meta,
            tc.tile_pool(name="upc", bufs=2) as upc,
            tc.tile_pool(name="seqpan", bufs=1) as seqpan,
            tc.tile_pool(name="xbuck", bufs=1) as xbuck,
        ):
            w_sb = cpool.tile([128, 2, OUT_FT], BF16)
            nc.sync.dma_start(out=w_sb[:], in_=w_in[:])
            bias_sb = cpool.tile([OUT_FT, 1], F32)
            nc.sync.dma_start(out=bias_sb[:], in_=bias_in[:])
            iota_sb = cpool.tile([128, nch * 128], F16)
            nc.sync.dma_start(out=iota_sb[:], in_=iota_in[:])

            # phase-2 meta, DMA'd FIRST so the sync queue serves these before
            # the upcast DMAs (which stall in-order on AllGather sems)
            WIN0, WIN1 = 3, 2
            state: dict[int, dict] = {}

            def load_meta(g, meta):
                gidx_sb = meta.tile([128, sgt * 8], I16, tag="gidx")
                nc.sync.dma_start(out=gidx_sb[:], in_=gidx[:, g])
                val_sb = meta.tile([128, sgt], F16, tag="val")
                nc.sync.dma_start(out=val_sb[:], in_=val[:, g])
                rl_sb = meta.tile([128, GB, nch], F16, tag="rl")
                nc.sync.dma_start(out=rl_sb[:],
                                  in_=rl[:, g * GB:(g + 1) * GB, :])
                state[g] = {"gidx": gidx_sb, "val": val_sb, "rl": rl_sb}

            for g in range(WIN0):
                load_meta(g, meta)

            # ---- phase 1: x = seq @ W (bf16 -> f32 psum) ----
            UPR = 4             # upcast slice: 128 x UPR rows at a time

            def project(b):
                pan = seqpan.tile([128, 2, QB[b] * 128], BF16, tag="pan",
                                  name="pan")
                for kc in range(2):
                    nc.sync.dma_start(
                        out=pan[:, kc, :],
                        in_=seqT[kc, :, QBS[b] * 128:QBS[b + 1] * 128])
                xb = xbuck.tile([128, QB[b], OUT_FT], F16, tag="xb",
                                name="xb")
                for j in range(QB[b]):
                    px = psum_pool.tile([128, OUT_FT], F32, tag="px",
                                        name="px")
                    for kc in range(2):
                        nc.tensor.matmul(
                            px[:],
                            pan[:, kc, j * 128:(j + 1) * 128],
                            w_sb[:, kc, :],
                            start=(kc == 0),
                            stop=(kc == 1),
                        )
                    nc.vector.tensor_copy(out=xb[:, j, :], in_=px[:])
                nc.sync.dma_start(out=ccin[b][:], in_=xb[:])
                nc.sync.dma_start(
                    out=sf_out[:, QBS[b]:QBS[b + 1], :], in_=xb[:])

            def allgather(b):
                nc.gpsimd.collective_compute(
                    "AllGather",
                    mybir.AluOpType.bypass,
                    replica_groups=groups,
                    ins=[ccin[b][:]],
                    outs=[xfull16[b][:]],
                )
                # upcast fp16 table slice-by-slice into the f32 gather
                # table. These DMAs ride the scalar engine's HWDGE queue:
                # on sync they would stall later DMAs behind this
                # AllGather's semaphore (in-order dispatch).
                nrow = SUBROWS[b] // 128
                for r0 in range(0, nrow, UPR):
                    rw = min(UPR, nrow - r0)
                    t16 = upc.tile([128, UPR, OUT_FT], F16, tag="t16",
                                   name="t16")
                    nc.scalar.dma_start(
                        out=t16[:, :rw, :],
                        in_=xfull16[b][r0 * 128:(r0 + rw) * 128, :]
                        .rearrange("(p c) f -> p c f", p=128))
                    t32 = upc.tile([128, UPR, OUT_FT], F32, tag="t32",
                                   name="t32")
                    nc.scalar.activation(
                        out=t32[:, :rw, :], in_=t16[:, :rw, :],
                        func=mybir.ActivationFunctionType.Copy)
                    nc.scalar.dma_start(
                        out=xfull[b][r0 * 128:(r0 + rw) * 128, :]
                        .rearrange("(p c) f -> p c f", p=128),
                        in_=t32[:, :rw, :])

            # ---- phase 2: bulk gather + scale + fold + one-hot matmul ----
            # The gpsimd engine resolves semaphore waits IN PROGRAM ORDER:
            # AllGather b and warmup gathers are interleaved so descriptor
            # generation for early buckets starts while later collectives
            # are still in flight.
            gq = [0]            # rotating SWDGE queue counter

            def gather_b(g, b, xgp, xg0p):
                st = state[g]
                ncols = GB * colsb[b]
                pool = xg0p if b == 0 else xgp
                tsz = ncols if b == 0 else GB * max(colsb[1:])
                xgt = pool.tile([128, tsz, 64], F32, tag=f"xg{b}",
                                name="xgt")
                xgb = xgt[:, :ncols, :]
                for off in range(0, ncols, MAXCH):
                    ln = min(MAXCH, ncols - off)
                    c0 = (int(gbase[b]) + off) * 8
                    nc.gpsimd.dma_gather(
                        out_ap=xgb[:, off:off + ln, :],
                        in_ap=xfull[b][:],
                        idxs_ap=st["gidx"][:, c0:c0 + ln * 8],
                        num_idxs=ln * 128,
                        num_idxs_reg=ln * 128,
                        elem_size=OUT_FT,
                        queue_num=gq[0] % NQ,
                    )
                    gq[0] += 1
                st[f"xg{b}"] = xgb

            ncols_max = GB * max(colsb)

            def scale_b(g, b, xgmp):
                st = state[g]
                ncols = GB * colsb[b]
                xgm = xgmp.tile([128, ncols_max, 64], F16, tag="xgm")
                nc.vector.tensor_tensor(
                    out=xgm[:, :ncols, :],
                    in0=st[f"xg{b}"],
                    in1=st["val"][:, int(gbase[b]):int(gbase[b]) + ncols]
                    .unsqueeze(2).broadcast_to([128, ncols, 64]),
                    op=mybir.AluOpType.mult,
                )
                st[f"xgm{b}"] = xgm[:, :ncols, :]
                del st[f"xg{b}"]

            def fold_b(g, b, xr):
                st = state[g]
                J, nchb = JB[b], NCHB[b]
                # [128, (j c s), f] -> per chunk c: reduce over s
                v = st[f"xgm{b}"].rearrange(
                    "p (j c s) f -> p j c f s", c=nchb, s=J)
                for c in range(nchb):
                    nc.vector.tensor_reduce(
                        out=xr[:, :, int(chb[b]) + c, :],
                        in_=v[:, :, c, :, :],
                        axis=mybir.AxisListType.X,
                        op=mybir.AluOpType.add,
                    )
                del st[f"xgm{b}"]

            with (
                tc.tile_pool(name="xg0p", bufs=3) as xg0p,
                tc.tile_pool(name="xgp", bufs=3) as xgp,
                tc.tile_pool(name="xgmp", bufs=3) as xgmp,
                tc.tile_pool(name="xrp", bufs=2) as xrp,
                tc.tile_pool(name="ohp", bufs=2) as ohp,
                tc.tile_pool(name="outp", bufs=2) as outp,
            ):
                for b in range(NSUB):
                    project(b)
                allgather(0)
                allgather(1)
                for g in range(WIN0):
                    gather_b(g, 0, xgp, xg0p)
                allgather(2)
                for g in range(WIN1):
                    gather_b(g, 1, xgp, xg0p)
                allgather(3)

                for g in range(NGROUPS):
                    if g >= WIN0:
                        load_meta(g, meta)
                        gather_b(g, 0, xgp, xg0p)
                    if g >= WIN1:
                        gather_b(g, 1, xgp, xg0p)
                    gather_b(g, 2, xgp, xg0p)
                    gather_b(g, 3, xgp, xg0p)
                    xr = xrp.tile([128, GB, nch, 64], F16, tag="xr")
                    with nc.allow_low_precision("f16 fold of <=6 f16 terms"):
                        for b in range(NSUB):
                            scale_b(g, b, xgmp)
                            fold_b(g, b, xr)
                    rl_sb = state[g]["rl"]
                    out_sb = outp.tile([OUT_FT, GB * 128], F16, tag="out_sb")
                    for j in range(GB):
                        # expand rowloc along q on the (otherwise idle)
                        # scalar engine, then a unit-stride is_equal on
                        # vector: A[p, c, q] = (rowloc[p, c] == q)
                        rlx = ohp.tile([128, nch * 128], F16, tag="rlx")
                        nc.scalar.activation(
                            out=rlx[:].rearrange("p (c q) -> p c q", q=128),
                            in_=rl_sb[:, j, :].unsqueeze(2).broadcast_to(
                                [128, nch, 128]),
                            func=mybir.ActivationFunctionType.Copy,
                        )
                        a_sb = ohp.tile([128, nch * 128], F16, tag="a_sb")
                        nc.vector.tensor_tensor(
                            out=a_sb[:],
                            in0=rlx[:],
                            in1=iota_sb[:],
                            op=mybir.AluOpType.is_equal,
                        )
                        po = psum_pool.tile([OUT_FT, 128], F32, tag="po")
                        for c in range(nch):
                            nc.tensor.matmul(
                                po[:],
                                xr[:, j, c, :],
                                a_sb[:, c * 128:(c + 1) * 128],
                                start=(c == 0),
                                stop=(c == nch - 1),
                            )
                        nc.scalar.activation(
                            out=out_sb[:, j * 128:(j + 1) * 128], in_=po[:],
                            func=mybir.ActivationFunctionType.Relu,
                            bias=bias_sb[:])
                    nc.sync.dma_start(
                        out=agg_out[:, g * GB * 128:(g + 1) * GB * 128],
                        in_=out_sb[:])
                    del state[g]

    nc.compile()
    return nc


def prepare_inputs(seq, edge_row, edge_col, edge_val, W, b):
    """Host-side sharding / graph partitioning. Returns (in_maps, cfg)."""
    import ml_dtypes

    seq = np.asarray(seq, dtype=np.float32).reshape(N_NODES, IN_FT)
    r = np.asarray(edge_row).astype(np.int64)
    c = np.asarray(edge_col).astype(np.int64)
    v = np.asarray(edge_val, dtype=np.float32)
    W = np.asarray(W, dtype=np.float32).reshape(IN_FT, OUT_FT)
    b = np.asarray(b, dtype=np.float32).reshape(OUT_FT)

    # bucket (by source block) of each source index
    blk_q = np.searchsorted(QBS[1:], np.arange(NB), side="right")
    qb_arr = np.asarray(QB)
    csrc = c // NS
    crem = c % NS
    cblk = crem // 128
    cp = crem % 128
    cq = blk_q[cblk]
    lidx = ((csrc * 128 + cp) * qb_arr[cq] + (cblk - QBS[cq])).astype(
        np.int16)

    core = r // NS
    loc = r - core * NS
    bucket = cq

    # per (core, node, bucket) degree
    D = np.zeros((NCORES, NS, NSUB), np.int64)
    np.add.at(D, (core, loc, bucket), 1)

    # pick first feasible config from the ladder
    for cfg in CFG_LADDER:
        JB, NCHB, colsb, chb, gbase, nch, sgt = cfg_geom(cfg)
        caps = 128 * np.asarray(NCHB, np.float64)
        LN = -(-D // np.asarray(JB)[None, None, :])   # lanes per node-bucket
        BLKA = np.empty((NCORES, NS), np.int32)
        ROWA = np.empty((NCORES, NS), np.int32)
        ok = True
        for k in range(NCORES):
            L = LN[k].astype(np.float64)
            order_n = np.argsort(-(L / caps[None]).sum(1), kind="stable")
            S = np.zeros((NB, NSUB))
            cnt = np.zeros(NB, np.int64)
            for n in order_n:
                d = L[n]
                load = ((S + d) / caps).max(1)
                load[cnt >= 128] = np.inf
                bsel = int(np.argmin(load))
                BLKA[k, n] = bsel
                ROWA[k, n] = cnt[bsel]
                S[bsel] += d
                cnt[bsel] += 1
            if (S > caps[None]).any():
                ok = False
                break
        if ok:
            break
    assert ok, "no feasible fold config"

    global _PERM
    _PERM = (BLKA, ROWA)
    blk = BLKA[core, loc].astype(np.int64)

    # lane base per (core, block, bucket, node): nodes ordered by ROWA
    J_of = np.asarray(JB)[bucket]
    LBASE = np.zeros((NCORES * NS, NSUB), np.int64)
    nb_of_node = BLKA.reshape(-1).astype(np.int64)
    row_of_node = ROWA.reshape(-1)
    key = (np.arange(NCORES).repeat(NS)) * NB + nb_of_node
    order = np.lexsort((row_of_node, key))
    keys = key[order]
    grp_first = np.r_[0, np.flatnonzero(keys[1:] != keys[:-1]) + 1]
    for bb in range(NSUB):
        lns = LN[:, :, bb].reshape(-1)[order]
        cs = np.cumsum(lns) - lns          # exclusive cumsum
        offset = np.zeros(len(cs), np.int64)
        offset[grp_first] = cs[grp_first]
        offset = np.maximum.accumulate(offset)
        LBASE[order, bb] = cs - offset
    LBASE = LBASE.reshape(NCORES, NS, NSUB)

    # per-edge position within its (core, node, bucket) group
    ekey = (core * NS + loc) * NSUB + bucket
    order_e = np.argsort(ekey, kind="stable")
    ekey_s = ekey[order_e]
    uniq, start_idx = np.unique(ekey_s, return_index=True)
    grp_start_e = np.zeros(len(ekey_s), np.int64)
    grp_start_e[start_idx] = start_idx
    grp_start_e = np.maximum.accumulate(grp_start_e)
    pos_s = np.arange(N_EDGES) - grp_start_e
    pos = np.empty(N_EDGES, np.int64)
    pos[order_e] = pos_s

    lane_local = pos // J_of
    jslot = pos - lane_local * J_of
    lane_glob = LBASE[core, loc, bucket] + lane_local
    chunk = lane_glob // 128
    p_lane = lane_glob - chunk * 128
    nchb_of = np.asarray(NCHB)[bucket]
    colsb_arr = np.asarray(colsb)
    gbase_arr = np.asarray(gbase[:NSUB])
    jj = blk % GB
    g = blk // GB
    col_in_group = gbase_arr[bucket] + (jj * nchb_of + chunk) * J_of + jslot

    # ---- emit gidx (16-wrapped + replicated), val, rl -----------------------
    # pad gather slots point at SPREAD-OUT rows (val=0 kills them): a single
    # shared pad row serializes the DMA engines on one 256B HBM address
    rng = np.random.default_rng(12345)
    gidx_arr = np.empty((NCORES, 16, NGROUPS, sgt * 8), np.int16)
    for b_ in range(NSUB):
        w0, w1 = int(gbase[b_]) * 8, int(gbase[b_ + 1]) * 8
        gidx_arr[:, :, :, w0:w1] = rng.integers(
            0, SUBROWS[b_], size=(NCORES, 16, NGROUPS, w1 - w0),
            dtype=np.int16)
    val_arr = np.zeros((NCORES, 128, NGROUPS, sgt), np.float16)
    rl_arr = np.full((NCORES, 128, NB, nch), -1.0, np.float16)

    I = col_in_group * 128 + p_lane
    gidx_arr[core, I % 16, g, I // 16] = lidx
    val_arr[core, p_lane, g, col_in_group] = v.astype(np.float16)
    chg = np.asarray(chb[:NSUB])[bucket] + chunk
    rl_arr[core, p_lane, blk, chg] = ROWA[core, loc].astype(np.float16)

    gidx_full = np.broadcast_to(
        gidx_arr[:, None], (NCORES, 8, 16, NGROUPS, sgt * 8))
    gidx_full = np.ascontiguousarray(
        gidx_full.reshape(NCORES, 128, NGROUPS, sgt * 8))

    biasT = np.ascontiguousarray(b.reshape(OUT_FT, 1))
    iotat = np.broadcast_to(
        np.tile(np.arange(128, dtype=np.float16), nch),
        (128, nch * 128)).copy()
    w3 = np.ascontiguousarray(
        W.reshape(2, 128, OUT_FT).transpose(1, 0, 2)).astype(
            ml_dtypes.bfloat16)  # [128, 2, OUT_FT]

    in_maps = []
    for k in range(NCORES):
        shard = np.zeros((NSP, IN_FT), np.float32)
        shard[:NS] = seq[k * NS:(k + 1) * NS]
        seqT_k = np.ascontiguousarray(shard.T).reshape(
            2, 128, NSP).astype(ml_dtypes.bfloat16)
        in_maps.append({
            "seqT": seqT_k,
            "gidx": gidx_full[k],
            "val": np.ascontiguousarray(val_arr[k]),
            "rl": np.ascontiguousarray(rl_arr[k]),
            "w": w3,
            "biasT": biasT,
            "iotat": iotat,
        })
    return in_maps, cfg


_PROGRAMS: dict[tuple, object] = {}
_PERM = None


def kernel(seq, edge_row, edge_col, edge_val, W, b):
    in_maps, cfg = prepare_inputs(seq, edge_row, edge_col, edge_val, W, b)
    key = (cfg[0], cfg[1])
    prog = _PROGRAMS.get(key)
    if prog is None:
        prog = _PROGRAMS[key] = build_program(cfg)
    res = run_bass_kernel_spmd(prog, in_maps, core_ids=list(range(NCORES)))

    def unshard_agg():
        BLKA, ROWA = _PERM
        parts = []
        for k in range(NCORES):
            aggT = np.asarray(res.results[k]["aggT"], dtype=np.float32)
            cols = BLKA[k].astype(np.int64) * 128 + ROWA[k]
            parts.append(aggT[:, cols].T)
        return np.concatenate(parts)[None]

    def unshard_sf():
        parts = [
            np.asarray(res.results[k]["sf"], dtype=np.float32)
            .transpose(1, 0, 2).reshape(NSP, OUT_FT)[:NS]
            for k in range(NCORES)
        ]
        return np.concatenate(parts)[None]

    return unshard_agg(), unshard_sf()


# revision 36
# speedup vs baseline: 1.1343x; 1.1343x over previous
"""GCN layer (dense projection + sparse neighbor aggregation) on 8 Trainium2
NeuronCores via Bass/Tile.

Strategy: shard nodes (and their incident edges, grouped by destination row)
across the 8 cores; replicate W/b; AllGather the projected node features in
f32 (4 bucket-aligned sub-collectives pipelined with a bf16 projection); per
128-row output block, bulk-gather source rows with DMAGatherAnt (int16
indices into 4 sub-tables of <=32k rows), scale by edge_val on DVE, fold
J same-destination slots per lane with tensor_reduce (bucket-pure tiers:
J=5 x1 chunk for bucket 0, J=3 x2 chunks for buckets 1-3 => 7 one-hot
matmuls per block instead of 18+), and segment-sum via transposed
assignment-matrix matmuls accumulated in PSUM [64 feats x 128 dests];
bias+ReLU fused in one scalar activation per block.
"""

import sys

if "/opt/trn_rl_repo" not in sys.path:
    sys.path.insert(0, "/opt/trn_rl_repo")

import numpy as np

import concourse.bass as bass
import concourse.mybir as mybir
import concourse.tile as tile
from concourse import bacc
from concourse.bass_utils import run_bass_kernel_spmd

N_NODES = 100000
N_EDGES = 1600000
IN_FT = 256
OUT_FT = 64
NCORES = 8
NS = N_NODES // NCORES          # 12500 nodes per core
NB = (NS + 127) // 128          # 98 row blocks per core
NSP = NB * 128                  # 12544 padded nodes per core
GB = 7                          # row blocks per pipeline group (98 = 14 * 7)
NGROUPS = NB // GB              # 14
NSUB = 4                        # source-block buckets (int16 idx: <=32k rows)
QB = [16, 28, 27, 27]           # source blocks per bucket (sums to NB);
                                # small first bucket -> AllGather 0 fires early
QBS = np.concatenate([[0], np.cumsum(QB)]).astype(np.int64)
SUBROWS = [NCORES * 128 * q for q in QB]

F32 = mybir.dt.float32
F16 = mybir.dt.float16
BF16 = mybir.dt.bfloat16
I16 = mybir.dt.int16

MAXCH = 8                       # 1024 indices = HW cap per dma_gather
NQ = 4                          # SWDGE queues (ucode max)

# ---- bucket-pure fold tiers -------------------------------------------------
# cfg = (J per bucket, chunks per bucket). caps[b] = 128 * nchb[b] lanes.
CFG_LADDER = [
    ((5, 3, 3, 3), (1, 2, 2, 2)),
    ((6, 4, 4, 4), (1, 2, 2, 2)),
]


def cfg_geom(cfg):
    JB, NCHB = cfg
    colsb = [J * n for J, n in zip(JB, NCHB)]       # cols per bucket
    chb = np.concatenate([[0], np.cumsum(NCHB)])    # chunk offset per bucket
    gb_cols = [GB * cb for cb in colsb]             # group cols per bucket
    gbase = np.concatenate([[0], np.cumsum(gb_cols)])  # group col base
    nch = int(chb[-1])                              # chunks per block (7)
    sgt = int(gbase[-1])                            # group cols (161)
    return JB, NCHB, colsb, chb, gbase, nch, sgt


def build_program(cfg):
    """One SPMD Bass program; all 8 cores run it on their own shards."""
    JB, NCHB, colsb, chb, gbase, nch, sgt = cfg_geom(cfg)

    nc = bacc.Bacc("TRN2", target_bir_lowering=False, debug=False,
                   num_devices=NCORES, num_swdge_queues=NQ)

    seqT = nc.dram_tensor("seqT", [2, 128, NSP], BF16, kind="ExternalInput")
    gidx = nc.dram_tensor("gidx", [128, NGROUPS, sgt * 8], I16,
                          kind="ExternalInput")
    val = nc.dram_tensor("val", [128, NGROUPS, sgt], F16,
                         kind="ExternalInput")
    rl = nc.dram_tensor("rl", [128, NB, nch], F16, kind="ExternalInput")
    w_in = nc.dram_tensor("w", [128, 2, OUT_FT], BF16, kind="ExternalInput")
    bias_in = nc.dram_tensor("biasT", [OUT_FT, 1], F32, kind="ExternalInput")
    iota_in = nc.dram_tensor("iotat", [128, nch * 128], F16,
                             kind="ExternalInput")
    # partition-major layouts; host un-permutes
    sf_out = nc.dram_tensor("sf", [128, NB, OUT_FT], F16,
                            kind="ExternalOutput")
    ccin = [nc.dram_tensor(f"ccin{b}", [128, QB[b], OUT_FT], F16)
            for b in range(NSUB)]
    agg_out = nc.dram_tensor("aggT", [OUT_FT, NB * 128], F16,
                             kind="ExternalOutput")
    # fp16 AllGather output; upcast on-device into the f32 gather table
    # (dma_gather elements must be a multiple of 256B = 64 x f32)
    xfull16 = [nc.dram_tensor(f"xfull16_{b}", [SUBROWS[b], OUT_FT], F16,
                              addr_space="Shared") for b in range(NSUB)]
    xfull = [nc.dram_tensor(f"xfull{b}", [SUBROWS[b], OUT_FT], F32)
             for b in range(NSUB)]

    groups = [list(range(NCORES))]

    with tile.TileContext(nc) as tc:
        with (
            tc.tile_pool(name="const", bufs=1) as cpool,
            tc.tile_pool(name="psum", bufs=4, space="PSUM") as psum_pool,
            tc.tile_pool(name="meta", bufs=5) as meta,
            tc.tile_pool(name="upc", bufs=2) as upc,
            tc.tile_pool(name="seqpan", bufs=1) as seqpan,
            tc.tile_pool(name="xbuck", bufs=2) as xbuck,
        ):
            w_sb = cpool.tile([128, 2, OUT_FT], BF16)
            nc.sync.dma_start(out=w_sb[:], in_=w_in[:])
            bias_sb = cpool.tile([OUT_FT, 1], F32)
            nc.sync.dma_start(out=bias_sb[:], in_=bias_in[:])
            iota_sb = cpool.tile([128, nch * 128], F16)
            nc.sync.dma_start(out=iota_sb[:], in_=iota_in[:])

            # phase-2 meta, DMA'd FIRST so the sync queue serves these before
            # the upcast DMAs (which stall in-order on AllGather sems)
            WIN0, WIN1 = 3, 2
            state: dict[int, dict] = {}

            def load_meta(g, meta):
                gidx_sb = meta.tile([128, sgt * 8], I16, tag="gidx")
                nc.sync.dma_start(out=gidx_sb[:], in_=gidx[:, g])
                val_sb = meta.tile([128, sgt], F16, tag="val")
                nc.sync.dma_start(out=val_sb[:], in_=val[:, g])
                rl_sb = meta.tile([128, GB, nch], F16, tag="rl")
                nc.sync.dma_start(out=rl_sb[:],
                                  in_=rl[:, g * GB:(g + 1) * GB, :])
                state[g] = {"gidx": gidx_sb, "val": val_sb, "rl": rl_sb}

            for g in range(WIN0):
                load_meta(g, meta)

            # ---- phase 1: x = seq @ W (bf16 -> f32 psum) ----
            UPR = 4             # upcast slice: 128 x UPR rows at a time

            def project(b):
                pan = seqpan.tile([128, 2, QB[b] * 128], BF16, tag="pan",
                                  name="pan")
                for kc in range(2):
                    nc.sync.dma_start(
                        out=pan[:, kc, :],
                        in_=seqT[kc, :, QBS[b] * 128:QBS[b + 1] * 128])
                xb = xbuck.tile([128, QB[b], OUT_FT], F16, tag="xb",
                                name="xb")
                for j in range(QB[b]):
                    px = psum_pool.tile([128, OUT_FT], F32, tag="px",
                                        name="px")
                    for kc in range(2):
                        nc.tensor.matmul(
                            px[:],
                            pan[:, kc, j * 128:(j + 1) * 128],
                            w_sb[:, kc, :],
                            start=(kc == 0),
                            stop=(kc == 1),
                        )
                    nc.vector.tensor_copy(out=xb[:, j, :], in_=px[:])
                nc.sync.dma_start(out=ccin[b][:], in_=xb[:])
                nc.sync.dma_start(
                    out=sf_out[:, QBS[b]:QBS[b + 1], :], in_=xb[:])

            def allgather(b):
                nc.gpsimd.collective_compute(
                    "AllGather",
                    mybir.AluOpType.bypass,
                    replica_groups=groups,
                    ins=[ccin[b][:]],
                    outs=[xfull16[b][:]],
                )
                # upcast fp16 table slice-by-slice into the f32 gather
                # table. These DMAs ride the scalar engine's HWDGE queue:
                # on sync they would stall later DMAs behind this
                # AllGather's semaphore (in-order dispatch).
                nrow = SUBROWS[b] // 128
                for r0 in range(0, nrow, UPR):
                    rw = min(UPR, nrow - r0)
                    t16 = upc.tile([128, UPR, OUT_FT], F16, tag="t16",
                                   name="t16")
                    nc.scalar.dma_start(
                        out=t16[:, :rw, :],
                        in_=xfull16[b][r0 * 128:(r0 + rw) * 128, :]
                        .rearrange("(p c) f -> p c f", p=128))
                    t32 = upc.tile([128, UPR, OUT_FT], F32, tag="t32",
                                   name="t32")
                    nc.scalar.activation(
                        out=t32[:, :rw, :], in_=t16[:, :rw, :],
                        func=mybir.ActivationFunctionType.Copy)
                    nc.scalar.dma_start(
                        out=xfull[b][r0 * 128:(r0 + rw) * 128, :]
                        .rearrange("(p c) f -> p c f", p=128),
                        in_=t32[:, :rw, :])

            # ---- phase 2: bulk gather + scale + fold + one-hot matmul ----
            # The gpsimd engine resolves semaphore waits IN PROGRAM ORDER:
            # AllGather b and warmup gathers are interleaved so descriptor
            # generation for early buckets starts while later collectives
            # are still in flight.
            gq = [0]            # rotating SWDGE queue counter

            def gather_b(g, b, xgp, xg0p):
                st = state[g]
                ncols = GB * colsb[b]
                pool = xg0p if b == 0 else xgp
                tsz = ncols if b == 0 else GB * max(colsb[1:])
                xgt = pool.tile([128, tsz, 64], F32, tag=f"xg{b}",
                                name="xgt")
                xgb = xgt[:, :ncols, :]
                for off in range(0, ncols, MAXCH):
                    ln = min(MAXCH, ncols - off)
                    c0 = (int(gbase[b]) + off) * 8
                    nc.gpsimd.dma_gather(
                        out_ap=xgb[:, off:off + ln, :],
                        in_ap=xfull[b][:],
                        idxs_ap=st["gidx"][:, c0:c0 + ln * 8],
                        num_idxs=ln * 128,
                        num_idxs_reg=ln * 128,
                        elem_size=OUT_FT,
                        queue_num=gq[0] % NQ,
                    )
                    gq[0] += 1
                st[f"xg{b}"] = xgb

            ncols_max = GB * max(colsb)

            def scale_b(g, b, xgmp):
                st = state[g]
                ncols = GB * colsb[b]
                xgm = xgmp.tile([128, ncols_max, 64], F16, tag="xgm")
                nc.vector.tensor_tensor(
                    out=xgm[:, :ncols, :],
                    in0=st[f"xg{b}"],
                    in1=st["val"][:, int(gbase[b]):int(gbase[b]) + ncols]
                    .unsqueeze(2).broadcast_to([128, ncols, 64]),
                    op=mybir.AluOpType.mult,
                )
                st[f"xgm{b}"] = xgm[:, :ncols, :]
                del st[f"xg{b}"]

            def fold_b(g, b, xr):
                st = state[g]
                J, nchb = JB[b], NCHB[b]
                # [128, (j c s), f] -> per chunk c: reduce over s
                v = st[f"xgm{b}"].rearrange(
                    "p (j c s) f -> p j c f s", c=nchb, s=J)
                for c in range(nchb):
                    nc.vector.tensor_reduce(
                        out=xr[:, :, int(chb[b]) + c, :],
                        in_=v[:, :, c, :, :],
                        axis=mybir.AxisListType.X,
                        op=mybir.AluOpType.add,
                    )
                del st[f"xgm{b}"]

            with (
                tc.tile_pool(name="xg0p", bufs=3) as xg0p,
                tc.tile_pool(name="xgp", bufs=3) as xgp,
                tc.tile_pool(name="xgmp", bufs=4) as xgmp,
                tc.tile_pool(name="xrp", bufs=2) as xrp,
                tc.tile_pool(name="ohp", bufs=2) as ohp,
                tc.tile_pool(name="outp", bufs=2) as outp,
            ):
                for b in range(NSUB):
                    project(b)
                allgather(0)
                allgather(1)
                for g in range(WIN0):
                    gather_b(g, 0, xgp, xg0p)
                allgather(2)
                for g in range(WIN1):
                    gather_b(g, 1, xgp, xg0p)
                allgather(3)

                for g in range(NGROUPS):
                    if g >= WIN0:
                        load_meta(g, meta)
                        gather_b(g, 0, xgp, xg0p)
                    if g >= WIN1:
                        gather_b(g, 1, xgp, xg0p)
                    gather_b(g, 2, xgp, xg0p)
                    gather_b(g, 3, xgp, xg0p)
                    xr = xrp.tile([128, GB, nch, 64], F16, tag="xr")
                    with nc.allow_low_precision("f16 fold of <=6 f16 terms"):
                        for b in range(NSUB):
                            scale_b(g, b, xgmp)
                            fold_b(g, b, xr)
                    rl_sb = state[g]["rl"]
                    out_sb = outp.tile([OUT_FT, GB * 128], F16, tag="out_sb")
                    for j in range(GB):
                        # expand rowloc along q on the (otherwise idle)
                        # scalar engine, then a unit-stride is_equal on
                        # vector: A[p, c, q] = (rowloc[p, c] == q)
                        rlx = ohp.tile([128, nch * 128], F16, tag="rlx")
                        nc.scalar.activation(
                            out=rlx[:].rearrange("p (c q) -> p c q", q=128),
                            in_=rl_sb[:, j, :].unsqueeze(2).broadcast_to(
                                [128, nch, 128]),
                            func=mybir.ActivationFunctionType.Copy,
                        )
                        a_sb = ohp.tile([128, nch * 128], F16, tag="a_sb")
                        nc.vector.tensor_tensor(
                            out=a_sb[:],
                            in0=rlx[:],
                            in1=iota_sb[:],
                            op=mybir.AluOpType.is_equal,
                        )
                        po = psum_pool.tile([OUT_FT, 128], F32, tag="po")
                        for c in range(nch):
                            nc.tensor.matmul(
                                po[:],
                                xr[:, j, c, :],
                                a_sb[:, c * 128:(c + 1) * 128],
                                start=(c == 0),
                                stop=(c == nch - 1),
                            )
                        nc.scalar.activation(
                            out=out_sb[:, j * 128:(j + 1) * 128], in_=po[:],
                            func=mybir.ActivationFunctionType.Relu,
                            bias=bias_sb[:])
                    nc.sync.dma_start(
                        out=agg_out[:, g * GB * 128:(g + 1) * GB * 128],
                        in_=out_sb[:])
                    del state[g]

    nc.compile()
    return nc


def prepare_inputs(seq, edge_row, edge_col, edge_val, W, b):
    """Host-side sharding / graph partitioning. Returns (in_maps, cfg)."""
    import ml_dtypes

    seq = np.asarray(seq, dtype=np.float32).reshape(N_NODES, IN_FT)
    r = np.asarray(edge_row).astype(np.int64)
    c = np.asarray(edge_col).astype(np.int64)
    v = np.asarray(edge_val, dtype=np.float32)
    W = np.asarray(W, dtype=np.float32).reshape(IN_FT, OUT_FT)
    b = np.asarray(b, dtype=np.float32).reshape(OUT_FT)

    # bucket (by source block) of each source index
    blk_q = np.searchsorted(QBS[1:], np.arange(NB), side="right")
    qb_arr = np.asarray(QB)
    csrc = c // NS
    crem = c % NS
    cblk = crem // 128
    cp = crem % 128
    cq = blk_q[cblk]
    lidx = ((csrc * 128 + cp) * qb_arr[cq] + (cblk - QBS[cq])).astype(
        np.int16)

    core = r // NS
    loc = r - core * NS
    bucket = cq

    # per (core, node, bucket) degree
    D = np.zeros((NCORES, NS, NSUB), np.int64)
    np.add.at(D, (core, loc, bucket), 1)

    # pick first feasible config from the ladder
    for cfg in CFG_LADDER:
        JB, NCHB, colsb, chb, gbase, nch, sgt = cfg_geom(cfg)
        caps = 128 * np.asarray(NCHB, np.float64)
        LN = -(-D // np.asarray(JB)[None, None, :])   # lanes per node-bucket
        BLKA = np.empty((NCORES, NS), np.int32)
        ROWA = np.empty((NCORES, NS), np.int32)
        ok = True
        for k in range(NCORES):
            L = LN[k].astype(np.float64)
            order_n = np.argsort(-(L / caps[None]).sum(1), kind="stable")
            S = np.zeros((NB, NSUB))
            cnt = np.zeros(NB, np.int64)
            for n in order_n:
                d = L[n]
                load = ((S + d) / caps).max(1)
                load[cnt >= 128] = np.inf
                bsel = int(np.argmin(load))
                BLKA[k, n] = bsel
                ROWA[k, n] = cnt[bsel]
                S[bsel] += d
                cnt[bsel] += 1
            if (S > caps[None]).any():
                ok = False
                break
        if ok:
            break
    assert ok, "no feasible fold config"

    global _PERM
    _PERM = (BLKA, ROWA)
    blk = BLKA[core, loc].astype(np.int64)

    # lane base per (core, block, bucket, node): nodes ordered by ROWA
    J_of = np.asarray(JB)[bucket]
    LBASE = np.zeros((NCORES * NS, NSUB), np.int64)
    nb_of_node = BLKA.reshape(-1).astype(np.int64)
    row_of_node = ROWA.reshape(-1)
    key = (np.arange(NCORES).repeat(NS)) * NB + nb_of_node
    order = np.lexsort((row_of_node, key))
    keys = key[order]
    grp_first = np.r_[0, np.flatnonzero(keys[1:] != keys[:-1]) + 1]
    for bb in range(NSUB):
        lns = LN[:, :, bb].reshape(-1)[order]
        cs = np.cumsum(lns) - lns          # exclusive cumsum
        offset = np.zeros(len(cs), np.int64)
        offset[grp_first] = cs[grp_first]
        offset = np.maximum.accumulate(offset)
        LBASE[order, bb] = cs - offset
    LBASE = LBASE.reshape(NCORES, NS, NSUB)

    # per-edge position within its (core, node, bucket) group
    ekey = (core * NS + loc) * NSUB + bucket
    order_e = np.argsort(ekey, kind="stable")
    ekey_s = ekey[order_e]
    uniq, start_idx = np.unique(ekey_s, return_index=True)
    grp_start_e = np.zeros(len(ekey_s), np.int64)
    grp_start_e[start_idx] = start_idx
    grp_start_e = np.maximum.accumulate(grp_start_e)
    pos_s = np.arange(N_EDGES) - grp_start_e
    pos = np.empty(N_EDGES, np.int64)
    pos[order_e] = pos_s

    lane_local = pos // J_of
    jslot = pos - lane_local * J_of
    lane_glob = LBASE[core, loc, bucket] + lane_local
    chunk = lane_glob // 128
    p_lane = lane_glob - chunk * 128
    nchb_of = np.asarray(NCHB)[bucket]
    colsb_arr = np.asarray(colsb)
    gbase_arr = np.asarray(gbase[:NSUB])
    jj = blk % GB
    g = blk // GB
    col_in_group = gbase_arr[bucket] + (jj * nchb_of + chunk) * J_of + jslot

    # ---- emit gidx (16-wrapped + replicated), val, rl -----------------------
    # pad gather slots point at SPREAD-OUT rows (val=0 kills them): a single
    # shared pad row serializes the DMA engines on one 256B HBM address
    rng = np.random.default_rng(12345)
    gidx_arr = np.empty((NCORES, 16, NGROUPS, sgt * 8), np.int16)
    for b_ in range(NSUB):
        w0, w1 = int(gbase[b_]) * 8, int(gbase[b_ + 1]) * 8
        gidx_arr[:, :, :, w0:w1] = rng.integers(
            0, SUBROWS[b_], size=(NCORES, 16, NGROUPS, w1 - w0),
            dtype=np.int16)
    val_arr = np.zeros((NCORES, 128, NGROUPS, sgt), np.float16)
    rl_arr = np.full((NCORES, 128, NB, nch), -1.0, np.float16)

    I = col_in_group * 128 + p_lane
    gidx_arr[core, I % 16, g, I // 16] = lidx
    val_arr[core, p_lane, g, col_in_group] = v.astype(np.float16)
    chg = np.asarray(chb[:NSUB])[bucket] + chunk
    rl_arr[core, p_lane, blk, chg] = ROWA[core, loc].astype(np.float16)

    gidx_full = np.broadcast_to(
        gidx_arr[:, None], (NCORES, 8, 16, NGROUPS, sgt * 8))
    gidx_full = np.ascontiguousarray(
        gidx_full.reshape(NCORES, 128, NGROUPS, sgt * 8))

    biasT = np.ascontiguousarray(b.reshape(OUT_FT, 1))
    iotat = np.broadcast_to(
        np.tile(np.arange(128, dtype=np.float16), nch),
        (128, nch * 128)).copy()
    w3 = np.ascontiguousarray(
        W.reshape(2, 128, OUT_FT).transpose(1, 0, 2)).astype(
            ml_dtypes.bfloat16)  # [128, 2, OUT_FT]

    in_maps = []
    for k in range(NCORES):
        shard = np.zeros((NSP, IN_FT), np.float32)
        shard[:NS] = seq[k * NS:(k + 1) * NS]
        seqT_k = np.ascontiguousarray(shard.T).reshape(
            2, 128, NSP).astype(ml_dtypes.bfloat16)
        in_maps.append({
            "seqT": seqT_k,
            "gidx": gidx_full[k],
            "val": np.ascontiguousarray(val_arr[k]),
            "rl": np.ascontiguousarray(rl_arr[k]),
            "w": w3,
            "biasT": biasT,
            "iotat": iotat,
        })
    return in_maps, cfg


_PROGRAMS: dict[tuple, object] = {}
_PERM = None


def kernel(seq, edge_row, edge_col, edge_val, W, b):
    in_maps, cfg = prepare_inputs(seq, edge_row, edge_col, edge_val, W, b)
    key = (cfg[0], cfg[1])
    prog = _PROGRAMS.get(key)
    if prog is None:
        prog = _PROGRAMS[key] = build_program(cfg)
    res = run_bass_kernel_spmd(prog, in_maps, core_ids=list(range(NCORES)))

    def unshard_agg():
        BLKA, ROWA = _PERM
        parts = []
        for k in range(NCORES):
            aggT = np.asarray(res.results[k]["aggT"], dtype=np.float32)
            cols = BLKA[k].astype(np.int64) * 128 + ROWA[k]
            parts.append(aggT[:, cols].T)
        return np.concatenate(parts)[None]

    def unshard_sf():
        parts = [
            np.asarray(res.results[k]["sf"], dtype=np.float32)
            .transpose(1, 0, 2).reshape(NSP, OUT_FT)[:NS]
            for k in range(NCORES)
        ]
        return np.concatenate(parts)[None]

    return unshard_agg(), unshard_sf()


# revision 37
# speedup vs baseline: 1.2919x; 1.1389x over previous
"""GCN layer (dense projection + sparse neighbor aggregation) on 8 Trainium2
NeuronCores via Bass/Tile.

Strategy: shard nodes (and their incident edges, grouped by destination row)
across the 8 cores; replicate W/b; AllGather the projected node features in
f32 (4 bucket-aligned sub-collectives pipelined with a bf16 projection); per
128-row output block, bulk-gather source rows with DMAGatherAnt (int16
indices into 4 sub-tables of <=32k rows), scale by edge_val on DVE, fold
J same-destination slots per lane with tensor_reduce (bucket-pure tiers:
J=5 x1 chunk for bucket 0, J=3 x2 chunks for buckets 1-3 => 7 one-hot
matmuls per block instead of 18+), and segment-sum via transposed
assignment-matrix matmuls accumulated in PSUM [64 feats x 128 dests];
bias+ReLU fused in one scalar activation per block.
"""

import sys

if "/opt/trn_rl_repo" not in sys.path:
    sys.path.insert(0, "/opt/trn_rl_repo")

import numpy as np

import concourse.bass as bass
import concourse.mybir as mybir
import concourse.tile as tile
from concourse import bacc
from concourse.bass_utils import run_bass_kernel_spmd

N_NODES = 100000
N_EDGES = 1600000
IN_FT = 256
OUT_FT = 64
NCORES = 8
NS = N_NODES // NCORES          # 12500 nodes per core
NB = (NS + 127) // 128          # 98 row blocks per core
NSP = NB * 128                  # 12544 padded nodes per core
GB = 7                          # row blocks per pipeline group (98 = 14 * 7)
NGROUPS = NB // GB              # 14
NSUB = 4                        # source-block buckets (int16 idx: <=32k rows)
QB = [16, 28, 27, 27]           # source blocks per bucket (sums to NB);
                                # small first bucket -> AllGather 0 fires early
QBS = np.concatenate([[0], np.cumsum(QB)]).astype(np.int64)
SUBROWS = [NCORES * 128 * q for q in QB]

F32 = mybir.dt.float32
F16 = mybir.dt.float16
BF16 = mybir.dt.bfloat16
I16 = mybir.dt.int16

MAXCH = 8                       # 1024 indices = HW cap per dma_gather
NQ = 4                          # SWDGE queues (ucode max)

# ---- bucket-pure fold tiers -------------------------------------------------
# cfg = (J per bucket, chunks per bucket). caps[b] = 128 * nchb[b] lanes.
CFG_LADDER = [
    ((5, 3, 3, 3), (1, 2, 2, 2)),
    ((6, 4, 4, 4), (1, 2, 2, 2)),
]


def cfg_geom(cfg):
    JB, NCHB = cfg
    colsb = [J * n for J, n in zip(JB, NCHB)]       # cols per bucket
    chb = np.concatenate([[0], np.cumsum(NCHB)])    # chunk offset per bucket
    gb_cols = [GB * cb for cb in colsb]             # group cols per bucket
    gbase = np.concatenate([[0], np.cumsum(gb_cols)])  # group col base
    nch = int(chb[-1])                              # chunks per block (7)
    sgt = int(gbase[-1])                            # group cols (161)
    return JB, NCHB, colsb, chb, gbase, nch, sgt


def build_program(cfg):
    """One SPMD Bass program; all 8 cores run it on their own shards."""
    JB, NCHB, colsb, chb, gbase, nch, sgt = cfg_geom(cfg)

    nc = bacc.Bacc("TRN2", target_bir_lowering=False, debug=False,
                   num_devices=NCORES, num_swdge_queues=NQ)

    seqT = nc.dram_tensor("seqT", [2, 128, NSP], BF16, kind="ExternalInput")
    gidx = nc.dram_tensor("gidx", [128, NGROUPS, sgt * 8], I16,
                          kind="ExternalInput")
    val = nc.dram_tensor("val", [128, NGROUPS, sgt], F16,
                         kind="ExternalInput")
    rl = nc.dram_tensor("rl", [128, NB, nch], F16, kind="ExternalInput")
    w_in = nc.dram_tensor("w", [128, 2, OUT_FT], BF16, kind="ExternalInput")
    bias_in = nc.dram_tensor("biasT", [OUT_FT, 1], F32, kind="ExternalInput")
    iota_in = nc.dram_tensor("iotat", [128, nch * 128], F16,
                             kind="ExternalInput")
    # partition-major layouts; host un-permutes
    sf_out = nc.dram_tensor("sf", [128, NB, OUT_FT], F16,
                            kind="ExternalOutput")
    ccin = [nc.dram_tensor(f"ccin{b}", [128, QB[b], OUT_FT], F16)
            for b in range(NSUB)]
    agg_out = nc.dram_tensor("aggT", [OUT_FT, NB * 128], F16,
                             kind="ExternalOutput")
    # fp16 AllGather output; upcast on-device into the f32 gather table
    # (dma_gather elements must be a multiple of 256B = 64 x f32)
    xfull16 = [nc.dram_tensor(f"xfull16_{b}", [SUBROWS[b], OUT_FT], F16,
                              addr_space="Shared") for b in range(NSUB)]
    xfull = [nc.dram_tensor(f"xfull{b}", [SUBROWS[b], OUT_FT], F32)
             for b in range(NSUB)]

    groups = [list(range(NCORES))]

    with tile.TileContext(nc) as tc:
        with (
            tc.tile_pool(name="const", bufs=1) as cpool,
            tc.tile_pool(name="psum", bufs=4, space="PSUM") as psum_pool,
            tc.tile_pool(name="meta", bufs=5) as meta,
            tc.tile_pool(name="upc", bufs=2) as upc,
            tc.tile_pool(name="seqpan", bufs=1) as seqpan,
            tc.tile_pool(name="xbuck", bufs=2) as xbuck,
        ):
            w_sb = cpool.tile([128, 2, OUT_FT], BF16)
            nc.sync.dma_start(out=w_sb[:], in_=w_in[:])
            bias_sb = cpool.tile([OUT_FT, 1], F32)
            nc.sync.dma_start(out=bias_sb[:], in_=bias_in[:])
            iota_sb = cpool.tile([128, nch * 128], F16)
            nc.sync.dma_start(out=iota_sb[:], in_=iota_in[:])

            # phase-2 meta, DMA'd FIRST so the sync queue serves these before
            # the upcast DMAs (which stall in-order on AllGather sems)
            WIN0, WIN1 = 3, 2
            state: dict[int, dict] = {}

            def load_meta(g, meta):
                gidx_sb = meta.tile([128, sgt * 8], I16, tag="gidx")
                nc.sync.dma_start(out=gidx_sb[:], in_=gidx[:, g])
                val_sb = meta.tile([128, sgt], F16, tag="val")
                nc.sync.dma_start(out=val_sb[:], in_=val[:, g])
                rl_sb = meta.tile([128, GB, nch], F16, tag="rl")
                nc.sync.dma_start(out=rl_sb[:],
                                  in_=rl[:, g * GB:(g + 1) * GB, :])
                state[g] = {"gidx": gidx_sb, "val": val_sb, "rl": rl_sb}

            for g in range(WIN0):
                load_meta(g, meta)

            # ---- phase 1: x = seq @ W (bf16 -> f32 psum) ----
            UPR = 16            # upcast slice: 128 x UPR rows at a time

            def project(b):
                pan = seqpan.tile([128, 2, QB[b] * 128], BF16, tag="pan",
                                  name="pan")
                for kc in range(2):
                    nc.sync.dma_start(
                        out=pan[:, kc, :],
                        in_=seqT[kc, :, QBS[b] * 128:QBS[b + 1] * 128])
                xb = xbuck.tile([128, QB[b], OUT_FT], F16, tag="xb",
                                name="xb")
                for j in range(QB[b]):
                    px = psum_pool.tile([128, OUT_FT], F32, tag="px",
                                        name="px")
                    for kc in range(2):
                        nc.tensor.matmul(
                            px[:],
                            pan[:, kc, j * 128:(j + 1) * 128],
                            w_sb[:, kc, :],
                            start=(kc == 0),
                            stop=(kc == 1),
                        )
                    nc.vector.tensor_copy(out=xb[:, j, :], in_=px[:])
                nc.sync.dma_start(out=ccin[b][:], in_=xb[:])
                nc.sync.dma_start(
                    out=sf_out[:, QBS[b]:QBS[b + 1], :], in_=xb[:])

            def allgather(b):
                nc.gpsimd.collective_compute(
                    "AllGather",
                    mybir.AluOpType.bypass,
                    replica_groups=groups,
                    ins=[ccin[b][:]],
                    outs=[xfull16[b][:]],
                )
                # upcast fp16 table slice-by-slice into the f32 gather
                # table. These DMAs ride the scalar engine's HWDGE queue:
                # on sync they would stall later DMAs behind this
                # AllGather's semaphore (in-order dispatch).
                nrow = SUBROWS[b] // 128
                for r0 in range(0, nrow, UPR):
                    rw = min(UPR, nrow - r0)
                    t16 = upc.tile([128, UPR, OUT_FT], F16, tag="t16",
                                   name="t16")
                    nc.scalar.dma_start(
                        out=t16[:, :rw, :],
                        in_=xfull16[b][r0 * 128:(r0 + rw) * 128, :]
                        .rearrange("(p c) f -> p c f", p=128))
                    t32 = upc.tile([128, UPR, OUT_FT], F32, tag="t32",
                                   name="t32")
                    nc.scalar.activation(
                        out=t32[:, :rw, :], in_=t16[:, :rw, :],
                        func=mybir.ActivationFunctionType.Copy)
                    nc.scalar.dma_start(
                        out=xfull[b][r0 * 128:(r0 + rw) * 128, :]
                        .rearrange("(p c) f -> p c f", p=128),
                        in_=t32[:, :rw, :])

            # ---- phase 2: bulk gather + scale + fold + one-hot matmul ----
            # The gpsimd engine resolves semaphore waits IN PROGRAM ORDER:
            # AllGather b and warmup gathers are interleaved so descriptor
            # generation for early buckets starts while later collectives
            # are still in flight.
            gq = [0]            # rotating SWDGE queue counter

            def gather_b(g, b, xgp, xg0p):
                st = state[g]
                ncols = GB * colsb[b]
                pool = xg0p if b == 0 else xgp
                tsz = ncols if b == 0 else GB * max(colsb[1:])
                xgt = pool.tile([128, tsz, 64], F32, tag=f"xg{b}",
                                name="xgt")
                xgb = xgt[:, :ncols, :]
                for off in range(0, ncols, MAXCH):
                    ln = min(MAXCH, ncols - off)
                    c0 = (int(gbase[b]) + off) * 8
                    nc.gpsimd.dma_gather(
                        out_ap=xgb[:, off:off + ln, :],
                        in_ap=xfull[b][:],
                        idxs_ap=st["gidx"][:, c0:c0 + ln * 8],
                        num_idxs=ln * 128,
                        num_idxs_reg=ln * 128,
                        elem_size=OUT_FT,
                        queue_num=gq[0] % NQ,
                    )
                    gq[0] += 1
                st[f"xg{b}"] = xgb

            ncols_max = GB * max(colsb)

            def scale_b(g, b, xgmp):
                st = state[g]
                ncols = GB * colsb[b]
                xgm = xgmp.tile([128, ncols_max, 64], F16, tag="xgm")
                nc.vector.tensor_tensor(
                    out=xgm[:, :ncols, :],
                    in0=st[f"xg{b}"],
                    in1=st["val"][:, int(gbase[b]):int(gbase[b]) + ncols]
                    .unsqueeze(2).broadcast_to([128, ncols, 64]),
                    op=mybir.AluOpType.mult,
                )
                st[f"xgm{b}"] = xgm[:, :ncols, :]
                del st[f"xg{b}"]

            def fold_b(g, b, xr):
                st = state[g]
                J, nchb = JB[b], NCHB[b]
                # [128, (j c s), f] -> per chunk c: reduce over s
                v = st[f"xgm{b}"].rearrange(
                    "p (j c s) f -> p j c f s", c=nchb, s=J)
                for c in range(nchb):
                    nc.vector.tensor_reduce(
                        out=xr[:, :, int(chb[b]) + c, :],
                        in_=v[:, :, c, :, :],
                        axis=mybir.AxisListType.X,
                        op=mybir.AluOpType.add,
                    )
                del st[f"xgm{b}"]

            with (
                tc.tile_pool(name="xg0p", bufs=3) as xg0p,
                tc.tile_pool(name="xgp", bufs=2) as xgp,
                tc.tile_pool(name="xgmp", bufs=4) as xgmp,
                tc.tile_pool(name="xrp", bufs=2) as xrp,
                tc.tile_pool(name="ohp", bufs=2) as ohp,
                tc.tile_pool(name="outp", bufs=2) as outp,
            ):
                for b in range(NSUB):
                    project(b)
                allgather(0)
                allgather(1)
                for g in range(WIN0):
                    gather_b(g, 0, xgp, xg0p)
                allgather(2)
                for g in range(WIN1):
                    gather_b(g, 1, xgp, xg0p)
                allgather(3)

                for g in range(NGROUPS):
                    if g >= WIN0:
                        load_meta(g, meta)
                        gather_b(g, 0, xgp, xg0p)
                    if g >= WIN1:
                        gather_b(g, 1, xgp, xg0p)
                    gather_b(g, 2, xgp, xg0p)
                    gather_b(g, 3, xgp, xg0p)
                    xr = xrp.tile([128, GB, nch, 64], F16, tag="xr")
                    with nc.allow_low_precision("f16 fold of <=6 f16 terms"):
                        for b in range(NSUB):
                            scale_b(g, b, xgmp)
                            fold_b(g, b, xr)
                    rl_sb = state[g]["rl"]
                    out_sb = outp.tile([OUT_FT, GB * 128], F16, tag="out_sb")
                    for j in range(GB):
                        # expand rowloc along q on the (otherwise idle)
                        # scalar engine, then a unit-stride is_equal on
                        # vector: A[p, c, q] = (rowloc[p, c] == q)
                        rlx = ohp.tile([128, nch * 128], F16, tag="rlx")
                        nc.scalar.activation(
                            out=rlx[:].rearrange("p (c q) -> p c q", q=128),
                            in_=rl_sb[:, j, :].unsqueeze(2).broadcast_to(
                                [128, nch, 128]),
                            func=mybir.ActivationFunctionType.Copy,
                        )
                        a_sb = ohp.tile([128, nch * 128], F16, tag="a_sb")
                        nc.vector.tensor_tensor(
                            out=a_sb[:],
                            in0=rlx[:],
                            in1=iota_sb[:],
                            op=mybir.AluOpType.is_equal,
                        )
                        po = psum_pool.tile([OUT_FT, 128], F32, tag="po")
                        for c in range(nch):
                            nc.tensor.matmul(
                                po[:],
                                xr[:, j, c, :],
                                a_sb[:, c * 128:(c + 1) * 128],
                                start=(c == 0),
                                stop=(c == nch - 1),
                            )
                        nc.scalar.activation(
                            out=out_sb[:, j * 128:(j + 1) * 128], in_=po[:],
                            func=mybir.ActivationFunctionType.Relu,
                            bias=bias_sb[:])
                    nc.sync.dma_start(
                        out=agg_out[:, g * GB * 128:(g + 1) * GB * 128],
                        in_=out_sb[:])
                    del state[g]

    nc.compile()
    return nc


def prepare_inputs(seq, edge_row, edge_col, edge_val, W, b):
    """Host-side sharding / graph partitioning. Returns (in_maps, cfg)."""
    import ml_dtypes

    seq = np.asarray(seq, dtype=np.float32).reshape(N_NODES, IN_FT)
    r = np.asarray(edge_row).astype(np.int64)
    c = np.asarray(edge_col).astype(np.int64)
    v = np.asarray(edge_val, dtype=np.float32)
    W = np.asarray(W, dtype=np.float32).reshape(IN_FT, OUT_FT)
    b = np.asarray(b, dtype=np.float32).reshape(OUT_FT)

    # bucket (by source block) of each source index
    blk_q = np.searchsorted(QBS[1:], np.arange(NB), side="right")
    qb_arr = np.asarray(QB)
    csrc = c // NS
    crem = c % NS
    cblk = crem // 128
    cp = crem % 128
    cq = blk_q[cblk]
    lidx = ((csrc * 128 + cp) * qb_arr[cq] + (cblk - QBS[cq])).astype(
        np.int16)

    core = r // NS
    loc = r - core * NS
    bucket = cq

    # per (core, node, bucket) degree
    D = np.zeros((NCORES, NS, NSUB), np.int64)
    np.add.at(D, (core, loc, bucket), 1)

    # pick first feasible config from the ladder
    for cfg in CFG_LADDER:
        JB, NCHB, colsb, chb, gbase, nch, sgt = cfg_geom(cfg)
        caps = 128 * np.asarray(NCHB, np.float64)
        LN = -(-D // np.asarray(JB)[None, None, :])   # lanes per node-bucket
        BLKA = np.empty((NCORES, NS), np.int32)
        ROWA = np.empty((NCORES, NS), np.int32)
        ok = True
        for k in range(NCORES):
            L = LN[k].astype(np.float64)
            order_n = np.argsort(-(L / caps[None]).sum(1), kind="stable")
            S = np.zeros((NB, NSUB))
            cnt = np.zeros(NB, np.int64)
            for n in order_n:
                d = L[n]
                load = ((S + d) / caps).max(1)
                load[cnt >= 128] = np.inf
                bsel = int(np.argmin(load))
                BLKA[k, n] = bsel
                ROWA[k, n] = cnt[bsel]
                S[bsel] += d
                cnt[bsel] += 1
            if (S > caps[None]).any():
                ok = False
                break
        if ok:
            break
    assert ok, "no feasible fold config"

    global _PERM
    _PERM = (BLKA, ROWA)
    blk = BLKA[core, loc].astype(np.int64)

    # lane base per (core, block, bucket, node): nodes ordered by ROWA
    J_of = np.asarray(JB)[bucket]
    LBASE = np.zeros((NCORES * NS, NSUB), np.int64)
    nb_of_node = BLKA.reshape(-1).astype(np.int64)
    row_of_node = ROWA.reshape(-1)
    key = (np.arange(NCORES).repeat(NS)) * NB + nb_of_node
    order = np.lexsort((row_of_node, key))
    keys = key[order]
    grp_first = np.r_[0, np.flatnonzero(keys[1:] != keys[:-1]) + 1]
    for bb in range(NSUB):
        lns = LN[:, :, bb].reshape(-1)[order]
        cs = np.cumsum(lns) - lns          # exclusive cumsum
        offset = np.zeros(len(cs), np.int64)
        offset[grp_first] = cs[grp_first]
        offset = np.maximum.accumulate(offset)
        LBASE[order, bb] = cs - offset
    LBASE = LBASE.reshape(NCORES, NS, NSUB)

    # per-edge position within its (core, node, bucket) group
    ekey = (core * NS + loc) * NSUB + bucket
    order_e = np.argsort(ekey, kind="stable")
    ekey_s = ekey[order_e]
    uniq, start_idx = np.unique(ekey_s, return_index=True)
    grp_start_e = np.zeros(len(ekey_s), np.int64)
    grp_start_e[start_idx] = start_idx
    grp_start_e = np.maximum.accumulate(grp_start_e)
    pos_s = np.arange(N_EDGES) - grp_start_e
    pos = np.empty(N_EDGES, np.int64)
    pos[order_e] = pos_s

    lane_local = pos // J_of
    jslot = pos - lane_local * J_of
    lane_glob = LBASE[core, loc, bucket] + lane_local
    chunk = lane_glob // 128
    p_lane = lane_glob - chunk * 128
    nchb_of = np.asarray(NCHB)[bucket]
    colsb_arr = np.asarray(colsb)
    gbase_arr = np.asarray(gbase[:NSUB])
    jj = blk % GB
    g = blk // GB
    col_in_group = gbase_arr[bucket] + (jj * nchb_of + chunk) * J_of + jslot

    # ---- emit gidx (16-wrapped + replicated), val, rl -----------------------
    # pad gather slots point at SPREAD-OUT rows (val=0 kills them): a single
    # shared pad row serializes the DMA engines on one 256B HBM address
    rng = np.random.default_rng(12345)
    gidx_arr = np.empty((NCORES, 16, NGROUPS, sgt * 8), np.int16)
    for b_ in range(NSUB):
        w0, w1 = int(gbase[b_]) * 8, int(gbase[b_ + 1]) * 8
        gidx_arr[:, :, :, w0:w1] = rng.integers(
            0, SUBROWS[b_], size=(NCORES, 16, NGROUPS, w1 - w0),
            dtype=np.int16)
    val_arr = np.zeros((NCORES, 128, NGROUPS, sgt), np.float16)
    rl_arr = np.full((NCORES, 128, NB, nch), -1.0, np.float16)

    I = col_in_group * 128 + p_lane
    gidx_arr[core, I % 16, g, I // 16] = lidx
    val_arr[core, p_lane, g, col_in_group] = v.astype(np.float16)
    chg = np.asarray(chb[:NSUB])[bucket] + chunk
    rl_arr[core, p_lane, blk, chg] = ROWA[core, loc].astype(np.float16)

    gidx_full = np.broadcast_to(
        gidx_arr[:, None], (NCORES, 8, 16, NGROUPS, sgt * 8))
    gidx_full = np.ascontiguousarray(
        gidx_full.reshape(NCORES, 128, NGROUPS, sgt * 8))

    biasT = np.ascontiguousarray(b.reshape(OUT_FT, 1))
    iotat = np.broadcast_to(
        np.tile(np.arange(128, dtype=np.float16), nch),
        (128, nch * 128)).copy()
    w3 = np.ascontiguousarray(
        W.reshape(2, 128, OUT_FT).transpose(1, 0, 2)).astype(
            ml_dtypes.bfloat16)  # [128, 2, OUT_FT]

    in_maps = []
    for k in range(NCORES):
        shard = np.zeros((NSP, IN_FT), np.float32)
        shard[:NS] = seq[k * NS:(k + 1) * NS]
        seqT_k = np.ascontiguousarray(shard.T).reshape(
            2, 128, NSP).astype(ml_dtypes.bfloat16)
        in_maps.append({
            "seqT": seqT_k,
            "gidx": gidx_full[k],
            "val": np.ascontiguousarray(val_arr[k]),
            "rl": np.ascontiguousarray(rl_arr[k]),
            "w": w3,
            "biasT": biasT,
            "iotat": iotat,
        })
    return in_maps, cfg


_PROGRAMS: dict[tuple, object] = {}
_PERM = None


def kernel(seq, edge_row, edge_col, edge_val, W, b):
    in_maps, cfg = prepare_inputs(seq, edge_row, edge_col, edge_val, W, b)
    key = (cfg[0], cfg[1])
    prog = _PROGRAMS.get(key)
    if prog is None:
        prog = _PROGRAMS[key] = build_program(cfg)
    res = run_bass_kernel_spmd(prog, in_maps, core_ids=list(range(NCORES)))

    def unshard_agg():
        BLKA, ROWA = _PERM
        parts = []
        for k in range(NCORES):
            aggT = np.asarray(res.results[k]["aggT"], dtype=np.float32)
            cols = BLKA[k].astype(np.int64) * 128 + ROWA[k]
            parts.append(aggT[:, cols].T)
        return np.concatenate(parts)[None]

    def unshard_sf():
        parts = [
            np.asarray(res.results[k]["sf"], dtype=np.float32)
            .transpose(1, 0, 2).reshape(NSP, OUT_FT)[:NS]
            for k in range(NCORES)
        ]
        return np.concatenate(parts)[None]

    return unshard_agg(), unshard_sf()


# revision 43
# speedup vs baseline: 1.4087x; 1.0904x over previous
"""GCN layer (dense projection + sparse neighbor aggregation) on 8 Trainium2
NeuronCores via Bass/Tile.

Strategy: shard nodes (and their incident edges, grouped by destination row)
across the 8 cores; replicate W/b; AllGather the projected node features in
f32 (4 bucket-aligned sub-collectives pipelined with a bf16 projection); per
128-row output block, bulk-gather source rows with DMAGatherAnt (int16
indices into 4 sub-tables of <=32k rows), scale by edge_val on DVE, fold
J same-destination slots per lane with tensor_reduce (bucket-pure tiers:
J=5 x1 chunk for bucket 0, J=3 x2 chunks for buckets 1-3 => 7 one-hot
matmuls per block instead of 18+), and segment-sum via transposed
assignment-matrix matmuls accumulated in PSUM [64 feats x 128 dests];
bias+ReLU fused in one scalar activation per block.
"""

import sys

if "/opt/trn_rl_repo" not in sys.path:
    sys.path.insert(0, "/opt/trn_rl_repo")

import numpy as np

import concourse.bass as bass
import concourse.mybir as mybir
import concourse.tile as tile
from concourse import bacc
from concourse.bass_utils import run_bass_kernel_spmd

N_NODES = 100000
N_EDGES = 1600000
IN_FT = 256
OUT_FT = 64
NCORES = 8
NS = N_NODES // NCORES          # 12500 nodes per core
NB = (NS + 127) // 128          # 98 row blocks per core
NSP = NB * 128                  # 12544 padded nodes per core
GB = 7                          # row blocks per pipeline group (98 = 14 * 7)
NGROUPS = NB // GB              # 14
NSUB = 4                        # source-block buckets (int16 idx: <=32k rows)
QB = [16, 28, 27, 27]           # source blocks per bucket (sums to NB);
                                # small first bucket -> AllGather 0 fires early
QBS = np.concatenate([[0], np.cumsum(QB)]).astype(np.int64)
SUBROWS = [NCORES * 128 * q for q in QB]

F32 = mybir.dt.float32
F16 = mybir.dt.float16
BF16 = mybir.dt.bfloat16
I16 = mybir.dt.int16

MAXCH = 8                       # 1024 indices = HW cap per dma_gather
NQ = 4                          # SWDGE queues (ucode max)

# ---- bucket-pure fold tiers -------------------------------------------------
# cfg = (J per bucket, chunks per bucket). caps[b] = 128 * nchb[b] lanes.
# b0 (low degree) uses classic J=1 columns: fold pads would cost more
# gather slots than the extra one-hot matmuls cost tensor time.
CFG_LADDER = [
    ((1, 3, 3, 3), (3, 2, 2, 2)),
    ((1, 3, 3, 3), (4, 2, 2, 2)),
    ((6, 4, 4, 4), (1, 2, 2, 2)),
]


def cfg_geom(cfg):
    JB, NCHB = cfg
    colsb = [J * n for J, n in zip(JB, NCHB)]       # cols per bucket
    chb = np.concatenate([[0], np.cumsum(NCHB)])    # chunk offset per bucket
    gb_cols = [GB * cb for cb in colsb]             # group cols per bucket
    gbase = np.concatenate([[0], np.cumsum(gb_cols)])  # group col base
    nch = int(chb[-1])                              # chunks per block (7)
    sgt = int(gbase[-1])                            # group cols (161)
    return JB, NCHB, colsb, chb, gbase, nch, sgt


def build_program(cfg):
    """One SPMD Bass program; all 8 cores run it on their own shards."""
    JB, NCHB, colsb, chb, gbase, nch, sgt = cfg_geom(cfg)

    nc = bacc.Bacc("TRN2", target_bir_lowering=False, debug=False,
                   num_devices=NCORES, num_swdge_queues=NQ)

    seqT = nc.dram_tensor("seqT", [2, 128, NSP], BF16, kind="ExternalInput")
    gidx = nc.dram_tensor("gidx", [128, NGROUPS, sgt * 8], I16,
                          kind="ExternalInput")
    val = nc.dram_tensor("val", [128, NGROUPS, sgt], F16,
                         kind="ExternalInput")
    rl = nc.dram_tensor("rl", [128, NB, nch], F16, kind="ExternalInput")
    w_in = nc.dram_tensor("w", [128, 2, OUT_FT], BF16, kind="ExternalInput")
    bias_in = nc.dram_tensor("biasT", [OUT_FT, 1], F32, kind="ExternalInput")
    iota_in = nc.dram_tensor("iotat", [128, nch * 128], F16,
                             kind="ExternalInput")
    # partition-major layouts; host un-permutes
    sf_out = nc.dram_tensor("sf", [128, NB, OUT_FT], F16,
                            kind="ExternalOutput")
    ccin = [nc.dram_tensor(f"ccin{b}", [128, QB[b], OUT_FT], F16)
            for b in range(NSUB)]
    agg_out = nc.dram_tensor("aggT", [OUT_FT, NB * 128], F16,
                             kind="ExternalOutput")
    # fp16 AllGather output; upcast on-device into the f32 gather table
    # (dma_gather elements must be a multiple of 256B = 64 x f32)
    xfull16 = [nc.dram_tensor(f"xfull16_{b}", [SUBROWS[b], OUT_FT], F16,
                              addr_space="Shared") for b in range(NSUB)]
    xfull = [nc.dram_tensor(f"xfull{b}", [SUBROWS[b], OUT_FT], F32)
             for b in range(NSUB)]

    groups = [list(range(NCORES))]

    with tile.TileContext(nc) as tc:
        with (
            tc.tile_pool(name="const", bufs=1) as cpool,
            tc.tile_pool(name="psum", bufs=4, space="PSUM") as psum_pool,
            tc.tile_pool(name="meta", bufs=5) as meta,
            tc.tile_pool(name="upc", bufs=2) as upc,
            tc.tile_pool(name="seqpan", bufs=1) as seqpan,
            tc.tile_pool(name="xbuck", bufs=2) as xbuck,
        ):
            w_sb = cpool.tile([128, 2, OUT_FT], BF16)
            nc.sync.dma_start(out=w_sb[:], in_=w_in[:])
            bias_sb = cpool.tile([OUT_FT, 1], F32)
            nc.sync.dma_start(out=bias_sb[:], in_=bias_in[:])
            iota_sb = cpool.tile([128, nch * 128], F16)
            nc.sync.dma_start(out=iota_sb[:], in_=iota_in[:])

            # phase-2 meta, DMA'd FIRST so the sync queue serves these before
            # the upcast DMAs (which stall in-order on AllGather sems)
            WIN0, WIN1 = 3, 2
            state: dict[int, dict] = {}

            def load_meta(g, meta):
                gidx_sb = meta.tile([128, sgt * 8], I16, tag="gidx")
                nc.sync.dma_start(out=gidx_sb[:], in_=gidx[:, g])
                val_sb = meta.tile([128, sgt], F16, tag="val")
                nc.sync.dma_start(out=val_sb[:], in_=val[:, g])
                rl_sb = meta.tile([128, GB, nch], F16, tag="rl")
                nc.sync.dma_start(out=rl_sb[:],
                                  in_=rl[:, g * GB:(g + 1) * GB, :])
                state[g] = {"gidx": gidx_sb, "val": val_sb, "rl": rl_sb}

            for g in range(WIN0):
                load_meta(g, meta)

            # ---- phase 1: x = seq @ W (bf16 -> f32 psum) ----
            UPR = 16            # upcast slice: 128 x UPR rows at a time

            def project(b):
                pan = seqpan.tile([128, 2, QB[b] * 128], BF16, tag="pan",
                                  name="pan")
                for kc in range(2):
                    nc.sync.dma_start(
                        out=pan[:, kc, :],
                        in_=seqT[kc, :, QBS[b] * 128:QBS[b + 1] * 128])
                xb = xbuck.tile([128, QB[b], OUT_FT], F16, tag="xb",
                                name="xb")
                for j in range(QB[b]):
                    px = psum_pool.tile([128, OUT_FT], F32, tag="px",
                                        name="px")
                    for kc in range(2):
                        nc.tensor.matmul(
                            px[:],
                            pan[:, kc, j * 128:(j + 1) * 128],
                            w_sb[:, kc, :],
                            start=(kc == 0),
                            stop=(kc == 1),
                        )
                    nc.vector.tensor_copy(out=xb[:, j, :], in_=px[:])
                nc.sync.dma_start(out=ccin[b][:], in_=xb[:])
                nc.sync.dma_start(
                    out=sf_out[:, QBS[b]:QBS[b + 1], :], in_=xb[:])

            def allgather(b):
                nc.gpsimd.collective_compute(
                    "AllGather",
                    mybir.AluOpType.bypass,
                    replica_groups=groups,
                    ins=[ccin[b][:]],
                    outs=[xfull16[b][:]],
                )
                # upcast fp16 table slice-by-slice into the f32 gather
                # table. These DMAs ride the scalar engine's HWDGE queue:
                # on sync they would stall later DMAs behind this
                # AllGather's semaphore (in-order dispatch).
                nrow = SUBROWS[b] // 128
                for r0 in range(0, nrow, UPR):
                    rw = min(UPR, nrow - r0)
                    t16 = upc.tile([128, UPR, OUT_FT], F16, tag="t16",
                                   name="t16")
                    nc.scalar.dma_start(
                        out=t16[:, :rw, :],
                        in_=xfull16[b][r0 * 128:(r0 + rw) * 128, :]
                        .rearrange("(p c) f -> p c f", p=128))
                    t32 = upc.tile([128, UPR, OUT_FT], F32, tag="t32",
                                   name="t32")
                    nc.scalar.activation(
                        out=t32[:, :rw, :], in_=t16[:, :rw, :],
                        func=mybir.ActivationFunctionType.Copy)
                    nc.scalar.dma_start(
                        out=xfull[b][r0 * 128:(r0 + rw) * 128, :]
                        .rearrange("(p c) f -> p c f", p=128),
                        in_=t32[:, :rw, :])

            # ---- phase 2: bulk gather + scale + fold + one-hot matmul ----
            # The gpsimd engine resolves semaphore waits IN PROGRAM ORDER:
            # AllGather b and warmup gathers are interleaved so descriptor
            # generation for early buckets starts while later collectives
            # are still in flight.
            gq = [0]            # rotating SWDGE queue counter

            def gather_b(g, b, xgp, xg0p):
                st = state[g]
                ncols = GB * colsb[b]
                pool = xg0p if b == 0 else xgp
                tsz = ncols if b == 0 else GB * max(colsb[1:])
                xgt = pool.tile([128, tsz, 64], F32, tag=f"xg{b}",
                                name="xgt")
                xgb = xgt[:, :ncols, :]
                for off in range(0, ncols, MAXCH):
                    ln = min(MAXCH, ncols - off)
                    c0 = (int(gbase[b]) + off) * 8
                    nc.gpsimd.dma_gather(
                        out_ap=xgb[:, off:off + ln, :],
                        in_ap=xfull[b][:],
                        idxs_ap=st["gidx"][:, c0:c0 + ln * 8],
                        num_idxs=ln * 128,
                        num_idxs_reg=ln * 128,
                        elem_size=OUT_FT,
                        queue_num=gq[0] % NQ,
                    )
                    gq[0] += 1
                st[f"xg{b}"] = xgb

            ncols_max = GB * max(colsb[1:])

            def scale_b(g, b, xgmp, xgm0p):
                st = state[g]
                ncols = GB * colsb[b]
                if b == 0:
                    xgm = xgm0p.tile([128, GB * colsb[0], 64], F16,
                                     tag="xgm0", name="xgm")
                else:
                    xgm = xgmp.tile([128, ncols_max, 64], F16, tag="xgm",
                                    name="xgm")
                nc.vector.tensor_tensor(
                    out=xgm[:, :ncols, :],
                    in0=st[f"xg{b}"],
                    in1=st["val"][:, int(gbase[b]):int(gbase[b]) + ncols]
                    .unsqueeze(2).broadcast_to([128, ncols, 64]),
                    op=mybir.AluOpType.mult,
                )
                st[f"xgm{b}"] = xgm[:, :ncols, :]
                del st[f"xg{b}"]

            def fold_b(g, b, xr):
                st = state[g]
                J, nchb = JB[b], NCHB[b]
                if J == 1:
                    return      # J=1 columns feed the matmul directly
                # [128, (j c s), f] -> per chunk c: reduce over s
                v = st[f"xgm{b}"].rearrange(
                    "p (j c s) f -> p j c f s", c=nchb, s=J)
                for c in range(nchb):
                    nc.vector.tensor_reduce(
                        out=xr[:, :, int(chb[b]) + c, :],
                        in_=v[:, :, c, :, :],
                        axis=mybir.AxisListType.X,
                        op=mybir.AluOpType.add,
                    )
                del st[f"xgm{b}"]

            with (
                tc.tile_pool(name="xg0p", bufs=3) as xg0p,
                tc.tile_pool(name="xgp", bufs=2) as xgp,
                tc.tile_pool(name="xgmp", bufs=4) as xgmp,
                tc.tile_pool(name="xgm0p", bufs=2) as xgm0p,
                tc.tile_pool(name="xrp", bufs=2) as xrp,
                tc.tile_pool(name="ohp", bufs=2) as ohp,
                tc.tile_pool(name="outp", bufs=2) as outp,
            ):
                for b in range(NSUB):
                    project(b)
                allgather(0)
                allgather(1)
                for g in range(WIN0):
                    gather_b(g, 0, xgp, xg0p)
                allgather(2)
                for g in range(WIN1):
                    gather_b(g, 1, xgp, xg0p)
                allgather(3)

                for g in range(NGROUPS):
                    if g >= WIN0:
                        load_meta(g, meta)
                        gather_b(g, 0, xgp, xg0p)
                    if g >= WIN1:
                        gather_b(g, 1, xgp, xg0p)
                    gather_b(g, 2, xgp, xg0p)
                    gather_b(g, 3, xgp, xg0p)
                    xr = xrp.tile([128, GB, nch, 64], F16, tag="xr")
                    with nc.allow_low_precision("f16 fold of <=6 f16 terms"):
                        for b in range(NSUB):
                            scale_b(g, b, xgmp, xgm0p)
                            fold_b(g, b, xr)
                    rl_sb = state[g]["rl"]
                    out_sb = outp.tile([OUT_FT, GB * 128], F16, tag="out_sb")
                    for j in range(GB):
                        # expand rowloc along q on the (otherwise idle)
                        # scalar engine, then a unit-stride is_equal on
                        # vector: A[p, c, q] = (rowloc[p, c] == q)
                        rlx = ohp.tile([128, nch * 128], F16, tag="rlx")
                        nc.scalar.activation(
                            out=rlx[:].rearrange("p (c q) -> p c q", q=128),
                            in_=rl_sb[:, j, :].unsqueeze(2).broadcast_to(
                                [128, nch, 128]),
                            func=mybir.ActivationFunctionType.Copy,
                        )
                        a_sb = ohp.tile([128, nch * 128], F16, tag="a_sb")
                        nc.vector.tensor_tensor(
                            out=a_sb[:],
                            in0=rlx[:],
                            in1=iota_sb[:],
                            op=mybir.AluOpType.is_equal,
                        )
                        po = psum_pool.tile([OUT_FT, 128], F32, tag="po")
                        for c in range(nch):
                            # J=1 buckets: scaled slots ARE the messages
                            if c < chb[1] and JB[0] == 1:
                                lhs = state[g]["xgm0"][
                                    :, j * NCHB[0] + c, :]
                            else:
                                lhs = xr[:, j, c, :]
                            nc.tensor.matmul(
                                po[:],
                                lhs,
                                a_sb[:, c * 128:(c + 1) * 128],
                                start=(c == 0),
                                stop=(c == nch - 1),
                            )
                        nc.scalar.activation(
                            out=out_sb[:, j * 128:(j + 1) * 128], in_=po[:],
                            func=mybir.ActivationFunctionType.Relu,
                            bias=bias_sb[:])
                    nc.sync.dma_start(
                        out=agg_out[:, g * GB * 128:(g + 1) * GB * 128],
                        in_=out_sb[:])
                    del state[g]

    nc.compile()
    return nc


def prepare_inputs(seq, edge_row, edge_col, edge_val, W, b):
    """Host-side sharding / graph partitioning. Returns (in_maps, cfg)."""
    import ml_dtypes

    seq = np.asarray(seq, dtype=np.float32).reshape(N_NODES, IN_FT)
    r = np.asarray(edge_row).astype(np.int64)
    c = np.asarray(edge_col).astype(np.int64)
    v = np.asarray(edge_val, dtype=np.float32)
    W = np.asarray(W, dtype=np.float32).reshape(IN_FT, OUT_FT)
    b = np.asarray(b, dtype=np.float32).reshape(OUT_FT)

    # bucket (by source block) of each source index
    blk_q = np.searchsorted(QBS[1:], np.arange(NB), side="right")
    qb_arr = np.asarray(QB)
    csrc = c // NS
    crem = c % NS
    cblk = crem // 128
    cp = crem % 128
    cq = blk_q[cblk]
    lidx = ((csrc * 128 + cp) * qb_arr[cq] + (cblk - QBS[cq])).astype(
        np.int16)

    core = r // NS
    loc = r - core * NS
    bucket = cq

    # per (core, node, bucket) degree
    D = np.zeros((NCORES, NS, NSUB), np.int64)
    np.add.at(D, (core, loc, bucket), 1)

    # pick first feasible config from the ladder
    for cfg in CFG_LADDER:
        JB, NCHB, colsb, chb, gbase, nch, sgt = cfg_geom(cfg)
        caps = 128 * np.asarray(NCHB, np.float64)
        LN = -(-D // np.asarray(JB)[None, None, :])   # lanes per node-bucket
        BLKA = np.empty((NCORES, NS), np.int32)
        ROWA = np.empty((NCORES, NS), np.int32)
        ok = True
        for k in range(NCORES):
            L = LN[k].astype(np.float64)
            order_n = np.argsort(-(L / caps[None]).sum(1), kind="stable")
            S = np.zeros((NB, NSUB))
            cnt = np.zeros(NB, np.int64)
            for n in order_n:
                d = L[n]
                load = ((S + d) / caps).max(1)
                load[cnt >= 128] = np.inf
                bsel = int(np.argmin(load))
                BLKA[k, n] = bsel
                ROWA[k, n] = cnt[bsel]
                S[bsel] += d
                cnt[bsel] += 1
            if (S > caps[None]).any():
                ok = False
                break
        if ok:
            break
    assert ok, "no feasible fold config"

    global _PERM
    _PERM = (BLKA, ROWA)
    blk = BLKA[core, loc].astype(np.int64)

    # lane base per (core, block, bucket, node): nodes ordered by ROWA
    J_of = np.asarray(JB)[bucket]
    LBASE = np.zeros((NCORES * NS, NSUB), np.int64)
    nb_of_node = BLKA.reshape(-1).astype(np.int64)
    row_of_node = ROWA.reshape(-1)
    key = (np.arange(NCORES).repeat(NS)) * NB + nb_of_node
    order = np.lexsort((row_of_node, key))
    keys = key[order]
    grp_first = np.r_[0, np.flatnonzero(keys[1:] != keys[:-1]) + 1]
    for bb in range(NSUB):
        lns = LN[:, :, bb].reshape(-1)[order]
        cs = np.cumsum(lns) - lns          # exclusive cumsum
        offset = np.zeros(len(cs), np.int64)
        offset[grp_first] = cs[grp_first]
        offset = np.maximum.accumulate(offset)
        LBASE[order, bb] = cs - offset
    LBASE = LBASE.reshape(NCORES, NS, NSUB)

    # per-edge position within its (core, node, bucket) group
    ekey = (core * NS + loc) * NSUB + bucket
    order_e = np.argsort(ekey, kind="stable")
    ekey_s = ekey[order_e]
    uniq, start_idx = np.unique(ekey_s, return_index=True)
    grp_start_e = np.zeros(len(ekey_s), np.int64)
    grp_start_e[start_idx] = start_idx
    grp_start_e = np.maximum.accumulate(grp_start_e)
    pos_s = np.arange(N_EDGES) - grp_start_e
    pos = np.empty(N_EDGES, np.int64)
    pos[order_e] = pos_s

    lane_local = pos // J_of
    jslot = pos - lane_local * J_of
    lane_glob = LBASE[core, loc, bucket] + lane_local
    chunk = lane_glob // 128
    p_lane = lane_glob - chunk * 128
    nchb_of = np.asarray(NCHB)[bucket]
    colsb_arr = np.asarray(colsb)
    gbase_arr = np.asarray(gbase[:NSUB])
    jj = blk % GB
    g = blk // GB
    col_in_group = gbase_arr[bucket] + (jj * nchb_of + chunk) * J_of + jslot

    # ---- emit gidx (16-wrapped + replicated), val, rl -----------------------
    # pad gather slots point at SPREAD-OUT rows (val=0 kills them): a single
    # shared pad row serializes the DMA engines on one 256B HBM address
    rng = np.random.default_rng(12345)
    gidx_arr = np.empty((NCORES, 16, NGROUPS, sgt * 8), np.int16)
    for b_ in range(NSUB):
        w0, w1 = int(gbase[b_]) * 8, int(gbase[b_ + 1]) * 8
        gidx_arr[:, :, :, w0:w1] = rng.integers(
            0, SUBROWS[b_], size=(NCORES, 16, NGROUPS, w1 - w0),
            dtype=np.int16)
    val_arr = np.zeros((NCORES, 128, NGROUPS, sgt), np.float16)
    rl_arr = np.full((NCORES, 128, NB, nch), -1.0, np.float16)

    I = col_in_group * 128 + p_lane
    gidx_arr[core, I % 16, g, I // 16] = lidx
    val_arr[core, p_lane, g, col_in_group] = v.astype(np.float16)
    chg = np.asarray(chb[:NSUB])[bucket] + chunk
    rl_arr[core, p_lane, blk, chg] = ROWA[core, loc].astype(np.float16)

    gidx_full = np.broadcast_to(
        gidx_arr[:, None], (NCORES, 8, 16, NGROUPS, sgt * 8))
    gidx_full = np.ascontiguousarray(
        gidx_full.reshape(NCORES, 128, NGROUPS, sgt * 8))

    biasT = np.ascontiguousarray(b.reshape(OUT_FT, 1))
    iotat = np.broadcast_to(
        np.tile(np.arange(128, dtype=np.float16), nch),
        (128, nch * 128)).copy()
    w3 = np.ascontiguousarray(
        W.reshape(2, 128, OUT_FT).transpose(1, 0, 2)).astype(
            ml_dtypes.bfloat16)  # [128, 2, OUT_FT]

    in_maps = []
    for k in range(NCORES):
        shard = np.zeros((NSP, IN_FT), np.float32)
        shard[:NS] = seq[k * NS:(k + 1) * NS]
        seqT_k = np.ascontiguousarray(shard.T).reshape(
            2, 128, NSP).astype(ml_dtypes.bfloat16)
        in_maps.append({
            "seqT": seqT_k,
            "gidx": gidx_full[k],
            "val": np.ascontiguousarray(val_arr[k]),
            "rl": np.ascontiguousarray(rl_arr[k]),
            "w": w3,
            "biasT": biasT,
            "iotat": iotat,
        })
    return in_maps, cfg


_PROGRAMS: dict[tuple, object] = {}
_PERM = None


def kernel(seq, edge_row, edge_col, edge_val, W, b):
    in_maps, cfg = prepare_inputs(seq, edge_row, edge_col, edge_val, W, b)
    key = (cfg[0], cfg[1])
    prog = _PROGRAMS.get(key)
    if prog is None:
        prog = _PROGRAMS[key] = build_program(cfg)
    res = run_bass_kernel_spmd(prog, in_maps, core_ids=list(range(NCORES)))

    def unshard_agg():
        BLKA, ROWA = _PERM
        parts = []
        for k in range(NCORES):
            aggT = np.asarray(res.results[k]["aggT"], dtype=np.float32)
            cols = BLKA[k].astype(np.int64) * 128 + ROWA[k]
            parts.append(aggT[:, cols].T)
        return np.concatenate(parts)[None]

    def unshard_sf():
        parts = [
            np.asarray(res.results[k]["sf"], dtype=np.float32)
            .transpose(1, 0, 2).reshape(NSP, OUT_FT)[:NS]
            for k in range(NCORES)
        ]
        return np.concatenate(parts)[None]

    return unshard_agg(), unshard_sf()
